# revision 68
# baseline (speedup 1.0000x reference)
"""Causal multi-head attention on 8 TRN2 NeuronCores — v3.

Problem: x[4, 2048, 768], 12 heads x d_head 64, causal softmax attention.
Sharding: core c handles batch b = c//2 and the 6-head group h0 = 6*(c%2);
host sums the two half-outputs per batch.

v3 changes over v2 (trace-driven; ~210us -> ~192us):
  - warmup: 8 full matmuls over independent psum chains (no
    complete-serialization) so the PE is continuously busy ~5-9us and
    HAM flips to 8/8 before real work, instead of a 7.4-17.6us
    serialized burst that blocked the projection stream until 17.6us.
  - input DMA: mask first, then wq/wk (+wv, wo) on the scalar hw queue
    in parallel with xt on the sync hw queue (two hardware DGE rings).
  - head: the k-outer now also computes K-pair1 n0/n1 (psz-pool banks
    are free in the head), and the K n2/n3 projections are woven into
    the hp0 t-loop after each supertile's QK stream is in flight, so
    the first attention matmul isn't pushed back.
  - main: QK pairs emitted in 2-j batches (fewer 64x128<->128x128
    mode-switch drains); AVs pumped in bursts of 12; deferred ZT scales
    drained mid-supertile; hp2 O-proj blocks interleaved into the next
    supertile's group loop instead of bunched at the boundary.
  - mask: one DVE tensor_tensor per diagonal j (3D AP over both heads)
    against a duplicated [128,256] mask tile, instead of two muls.
  - tail: all 4 O-proj blocks of the second-to-last supertile held back
    as PE filler under the final normalize chain; final normalize reads
    z straight from psum with split muls; tail psum drains go through
    the tail-idle scalar engine.
"""

import sys

if "/opt/trn_rl_repo" not in sys.path:
    sys.path.insert(0, "/opt/trn_rl_repo")

import numpy as np
import ml_dtypes

BF16NP = ml_dtypes.bfloat16


def _ensure_ntff_hook():
    import types
    if "antenv.axon_hooks" in sys.modules:
        return
    try:
        from trn_agent_boot.trn_boot import _ntff_profile_via_ctypes
        hook = _ntff_profile_via_ctypes("/opt/axon/libaxon_pjrt.so")
    except Exception:
        hook = None
    m = types.ModuleType("antenv.axon_hooks")
    m._hook = hook
    m.get_axon_ntff_profile_hook = lambda: m._hook
    def _set(h):
        m._hook = h
    m.set_axon_ntff_profile_hook = _set
    sys.modules["antenv.axon_hooks"] = m


_ensure_ntff_hook()

import concourse.bass as bass
import concourse.tile as tile
from concourse import bacc, mybir, library_config
from concourse.bass_utils import run_bass_kernel_spmd

F32 = mybir.dt.float32
BF16 = mybir.dt.bfloat16
AF = mybir.ActivationFunctionType

D = 768          # d_model
S = 2048         # seq
E = 64           # d_head
NHC = 6          # heads per core
HE = NHC * E     # 384
KD = D // 128    # 6 k-chunks over d_model
B = 4

LAST_EXEC_TIME_NS = None
_GRAPH_CACHE = {}


def _build_graph(qkv_bias: bool) -> bass.Bass:
    nc = bacc.Bacc("TRN2", target_bir_lowering=False)
    xt = nc.declare_dram_parameter("xt", [D, S], BF16, isOutput=False)
    wq = nc.declare_dram_parameter("wq", [D, HE], BF16, isOutput=False)
    wk = nc.declare_dram_parameter("wk", [D, HE], BF16, isOutput=False)
    wv = nc.declare_dram_parameter("wv", [D, HE], BF16, isOutput=False)
    wo = nc.declare_dram_parameter("wo", [HE, D], BF16, isOutput=False)
    mask2 = nc.declare_dram_parameter("mask2", [128, 256], BF16, isOutput=False)
    if qkv_bias:
        bq = nc.declare_dram_parameter("bq", [HE, 1], F32, isOutput=False)
        bk = nc.declare_dram_parameter("bk", [HE, 1], F32, isOutput=False)
        bv = nc.declare_dram_parameter("bv", [1, HE], BF16, isOutput=False)
    out = nc.declare_dram_parameter("out", [S, D], BF16, isOutput=True)

    with tile.TileContext(nc) as tc:
        with tc.tile_pool(name="persist", bufs=1) as persist, \
             tc.tile_pool(name="workE", bufs=24) as workE, \
             tc.tile_pool(name="workZ", bufs=6) as workZ, \
             tc.tile_pool(name="work2", bufs=3) as work2, \
             tc.tile_pool(name="workO", bufs=3) as workO, \
             tc.tile_pool(name="psS", bufs=2, space="PSUM") as psS, \
             tc.tile_pool(name="psZ", bufs=1, space="PSUM") as psZ, \
             tc.tile_pool(name="aux", bufs=2, space="PSUM") as auxP:

            QT = [persist.tile([128, S], BF16, tag=f"qt{m}", name=f"qt{m}") for m in range(3)]
            KT = [persist.tile([128, S], BF16, tag=f"kt{m}", name=f"kt{m}") for m in range(3)]
            ZT = [persist.tile([128, S], BF16, tag=f"zt{m}", name=f"zt{m}") for m in range(3)]
            VA = [persist.tile([128, NHC * 65], BF16, tag=f"va{s}", name=f"va{s}") for s in range(16)]
            WO = [persist.tile([128, D], BF16, tag=f"wo{m}", name=f"wo{m}") for m in range(3)]
            MSK2 = persist.tile([128, 256], BF16, tag="mask2", name="mask2_sb")
            XT = [persist.tile([128, S], BF16, tag=f"xt{k}", name=f"xt{k}") for k in range(KD)]
            WQs = [persist.tile([128, HE], BF16, tag=f"wq{k}", name=f"wq{k}") for k in range(KD)]
            WKs = [persist.tile([128, HE], BF16, tag=f"wk{k}", name=f"wk{k}") for k in range(KD)]
            WVs = [persist.tile([128, HE], BF16, tag=f"wv{k}", name=f"wv{k}") for k in range(KD)]

            nc.gpsimd.load_library(library_config.attn)

            # HAM warmup: 12 full-width matmuls over 6 independent psum
            # chains (2 per chain, interleaved) — continuous PE activity
            # ~5-10.5us with no wait-for-complete serialization, so the
            # clock gate is at 8/8 by the time the projections start.
            WUP = persist.tile([128, 512], BF16, tag="wup", name="wup")
            with tc.high_priority():
                nc.vector.memset(WUP[:], 0.0)
                for r in range(2):
                    for i in range(2):
                        pw = psS.tile([128, 1024], F32, tag="pss", name="pss_w")
                        nc.tensor.matmul(pw[:, 0:512], WUP[:, 0:128], WUP[:],
                                         start=True, stop=True)
                        pa = auxP.tile([128, 512], F32, tag="aux", name="aux_w")
                        nc.tensor.matmul(pa[:], WUP[:, 0:128], WUP[:],
                                         start=True, stop=True)

            # k-outer-critical inputs stay interleaved on the sync queue
            # (keeps packet aggregation + chunk ordering); everything not
            # needed until later loads in parallel on the other DMA queues
            nc.scalar.dma_start(out=MSK2[:], in_=mask2[:])
            for k in range(KD):
                nc.scalar.dma_start(out=WQs[k][:], in_=wq[k * 128:(k + 1) * 128, :])
                nc.scalar.dma_start(out=WKs[k][:], in_=wk[k * 128:(k + 1) * 128, :])
                nc.sync.dma_start(out=XT[k][:], in_=xt[k * 128:(k + 1) * 128, :])
            for k in range(KD):
                nc.scalar.dma_start(out=WVs[k][:], in_=wv[k * 128:(k + 1) * 128, :])
            for m in range(3):
                nc.gpsimd.dma_start(out=WO[m][:], in_=wo[m * 128:(m + 1) * 128, :])
            ONES = persist.tile([1, 128], BF16, tag="ones", name="ones_sb")
            nc.vector.memset(ONES[:], 1.0)
            if qkv_bias:
                BQ = persist.tile([128, 3], F32, tag="bq", name="bq_sb")
                BK = persist.tile([128, 3], F32, tag="bk", name="bk_sb")
                BV = persist.tile([1, HE], BF16, tag="bv", name="bv_sb")
                for m in range(3):
                    nc.sync.dma_start(out=BQ[:, m:m + 1], in_=bq[m * 128:(m + 1) * 128, :])
                    nc.sync.dma_start(out=BK[:, m:m + 1], in_=bk[m * 128:(m + 1) * 128, :])
                nc.sync.dma_start(out=BV[:], in_=bv[:])

            # ---------- emission helpers ----------
            def proj_q(hp, n):
                """Q^T slice [128, 512] for pair hp, q-supertile n."""
                ps = auxP.tile([128, 512], F32, tag="aux", name="ps_aux")
                for k in range(KD):
                    nc.tensor.matmul(
                        ps[:],
                        WQs[k][:, hp * 128:(hp + 1) * 128],
                        XT[k][:, n * 512:(n + 1) * 512],
                        start=(k == 0), stop=(k == KD - 1))
                dst = QT[hp][:, n * 512:(n + 1) * 512]
                if qkv_bias:
                    nc.scalar.activation(dst, ps[:], AF.Copy, bias=BQ[:, hp:hp + 1])
                else:
                    nc.vector.tensor_copy(dst, ps[:])

            def proj_qk(hp, n):
                """Q^T and K^T slice [128, 512] for pair hp, q-supertile n."""
                for Wt, Ot, bt in ((WQs, QT, "bq"), (WKs, KT, "bk")):
                    ps = auxP.tile([128, 512], F32, tag="aux", name="ps_aux")
                    for k in range(KD):
                        nc.tensor.matmul(
                            ps[:],
                            Wt[k][:, hp * 128:(hp + 1) * 128],
                            XT[k][:, n * 512:(n + 1) * 512],
                            start=(k == 0), stop=(k == KD - 1))
                    dst = Ot[hp][:, n * 512:(n + 1) * 512]
                    if qkv_bias:
                        bias_t = BQ if bt == "bq" else BK
                        nc.scalar.activation(dst, ps[:], AF.Copy,
                                             bias=bias_t[:, hp:hp + 1])
                    else:
                        nc.vector.tensor_copy(dst, ps[:])

            v_done = [False] * 16

            def proj_v(sc):
                """V rows for token block sc, all 6 heads, into VA[sc]."""
                if v_done[sc]:
                    return
                v_done[sc] = True
                nc.vector.memset(VA[sc][:], 1.0)
                ps = auxP.tile([128, 512], F32, tag="aux", name="ps_aux")
                pv = ps[:, 0:HE]
                for k in range(KD):
                    nc.tensor.matmul(
                        pv,
                        XT[k][:, sc * 128:(sc + 1) * 128],
                        WVs[k][:],
                        start=(k == 0), stop=False if qkv_bias else (k == KD - 1))
                if qkv_bias:
                    nc.tensor.matmul(pv, ONES[:], BV[:], start=False, stop=True)
                nc.vector.tensor_copy(
                    VA[sc][:].rearrange("p (h c) -> p h c", c=65)[:, :, 0:64],
                    pv.rearrange("p (h c) -> p h c", c=64))

            def phase_e_block(mc, act_copy=False):
                """Output projection for one 128-token block.  act_copy
                routes the psum drain through the (tail-idle) scalar engine
                so the DVE isn't the serial resource at the end."""
                ob = workO.tile([128, D], BF16, tag="ob", name="ob")
                for half in range(2):
                    po = auxP.tile([128, 512], F32, tag="aux", name="po")
                    pon = po[:, 0:HE]
                    n0 = half * HE
                    for k in range(3):
                        nc.tensor.matmul(
                            pon,
                            ZT[k][:, mc * 128:(mc + 1) * 128],
                            WO[k][:, n0:n0 + HE],
                            start=(k == 0), stop=(k == 2))
                    if act_copy:
                        nc.scalar.activation(ob[:, n0:n0 + HE], pon, AF.Copy)
                        nc.sync.dma_start(
                            out=out[mc * 128:(mc + 1) * 128, n0:n0 + HE],
                            in_=ob[:, n0:n0 + HE])
                    else:
                        nc.vector.tensor_copy(ob[:, n0:n0 + HE], pon)
                if not act_copy:
                    nc.sync.dma_start(out=out[mc * 128:(mc + 1) * 128, :], in_=ob[:])

            def phase_e(t):
                for mc in range(4 * t, 4 * t + 4):
                    phase_e_block(mc)

            # ---------- head phase ----------
            def head_phase():
                """Pair-0 Q (4 supertiles) + K (n=0,1) with the k-chunk loop
                outermost so each arriving (wq[k], wk[k], xt[k]) chunk
                unlocks 6 matmuls.  Afterwards (still inside the DMA-in
                window) re-stream K pair-0 n=2,3 and all of K pair-1, so
                the main phase only needs Q for pair 1."""
                pq = [psS.tile([128, 1024], F32, tag="pss", name="ps_q")
                      for _ in range(2)]
                pk = [auxP.tile([128, 512], F32, tag="aux", name="ps_k")
                      for _ in range(2)]
                pk1 = [psZ.tile([128, 512], F32, tag=f"pz{i}", name="ps_k1h")
                       for i in range(2)]
                for k in range(KD):
                    for n in range(4):
                        nc.tensor.matmul(
                            pq[n // 2][:, (n % 2) * 512:(n % 2) * 512 + 512],
                            WQs[k][:, 0:128],
                            XT[k][:, n * 512:(n + 1) * 512],
                            start=(k == 0), stop=(k == KD - 1))
                    for n in range(2):
                        nc.tensor.matmul(
                            pk[n][:],
                            WKs[k][:, 0:128],
                            XT[k][:, n * 512:(n + 1) * 512],
                            start=(k == 0), stop=(k == KD - 1))
                        nc.tensor.matmul(
                            pk1[n][:],
                            WKs[k][:, 128:256],
                            XT[k][:, n * 512:(n + 1) * 512],
                            start=(k == 0), stop=(k == KD - 1))
                if qkv_bias:
                    for i in range(2):
                        nc.scalar.activation(QT[0][:, i * 1024:(i + 1) * 1024],
                                             pq[i][:], AF.Copy, bias=BQ[:, 0:1])
                        nc.scalar.activation(KT[0][:, i * 512:(i + 1) * 512],
                                             pk[i][:], AF.Copy, bias=BK[:, 0:1])
                        nc.scalar.activation(KT[1][:, i * 512:(i + 1) * 512],
                                             pk1[i][:], AF.Copy, bias=BK[:, 1:2])
                else:
                    # t0's working set first, small and on the idle DVE, so
                    # the first QK group isn't gated by the big ACT copies
                    nc.vector.tensor_copy(QT[0][:, 0:512], pq[0][:, 0:512])
                    nc.vector.tensor_copy(KT[0][:, 0:512], pk[0][:])
                    nc.vector.tensor_copy(QT[0][:, 512:1024], pq[0][:, 512:1024])
                    nc.scalar.activation(QT[0][:, 1024:2048], pq[1][:], AF.Copy)
                    nc.scalar.activation(KT[0][:, 512:1024], pk[1][:], AF.Copy)
                    for i in range(2):
                        nc.scalar.activation(KT[1][:, i * 512:(i + 1) * 512],
                                             pk1[i][:], AF.Copy)
            def proj_k_single(hp, n):
                """K^T slice [128, 512] for pair hp, supertile n (aux psum)."""
                ps = auxP.tile([128, 512], F32, tag="aux", name="ps_aux")
                for k in range(KD):
                    nc.tensor.matmul(
                        ps[:], WKs[k][:, hp * 128:(hp + 1) * 128],
                        XT[k][:, n * 512:(n + 1) * 512],
                        start=(k == 0), stop=(k == KD - 1))
                dst = KT[hp][:, n * 512:(n + 1) * 512]
                if qkv_bias:
                    nc.scalar.activation(dst, ps[:], AF.Copy, bias=BK[:, hp:hp + 1])
                else:
                    nc.vector.tensor_copy(dst, ps[:])

            # ---------- main phase ----------
            pending = []   # deferred normalize tails
            av_q = []      # deferred AV matmuls: (av_fn, norm_fn|None)
            AV_LAG = 12

            def drain_pending(upto):
                while len(pending) > upto:
                    pending.pop(0)()

            def pump_avs(lag):
                while len(av_q) > lag:
                    av_fn, norm_fn = av_q.pop(0)
                    av_fn()
                    if norm_fn is not None:
                        norm_fn()
                        drain_pending(1)

            held_blocks = []  # O-proj blocks deferred as tail filler

            for hp in range(3):
                if hp == 0:
                    head_phase()
                t_order = (3, 0, 1, 2) if hp == 2 else (0, 1, 2, 3)
                for ti, t in enumerate(t_order):
                    weave_todo = []
                    if hp == 0:
                        k_weave = ((0, 2), (0, 3), (1, 2), (1, 3))[ti]
                        weave_todo = [lambda t=t: proj_q(1, t),
                                      lambda k_weave=k_weave:
                                      proj_k_single(*k_weave)]
                    elif hp == 1:
                        # spread pair-2 projections into the group loop so
                        # they fill PE slots while exps pace the QK stream
                        weave_todo = [lambda t=t: proj_q(2, t),
                                      lambda t=t: proj_k_single(2, t)]
                    pe_todo = []
                    if hp == 2 and ti >= 1:
                        pump_avs(0)
                        drain_pending(0)
                        tprev = t_order[ti - 1]
                        if ti == 3:
                            # hold all 4 blocks back to cover the final
                            # normalize latency after the last AVs
                            held_blocks.extend(range(4 * tprev, 4 * tprev + 4))
                        else:
                            # interleave the O-proj blocks into the j-group
                            # loop so the new supertile's QK stream runs
                            # while the DVE finishes the ZT scales
                            pe_todo = list(range(4 * tprev, 4 * tprev + 4))
                    psz = {}
                    final = (hp == 2 and ti == 3)

                    def emit_normalize(t=t, hp=hp, psz=psz, final=final):
                        """Both heads' psz done: drain, reciprocal, scale."""
                        if final:
                            # latency-optimized: scale straight from psum;
                            # broadcast via a K=1 PE outer-product (fast and
                            # off the busy DVE/gpsimd queues); muls split so
                            # O-proj blocks unblock progressively
                            den = work2.tile([1, 1024], F32, tag="den", name="den")
                            for par in (0, 1):
                                nc.vector.tensor_copy(
                                    den[:, par * 512:(par + 1) * 512],
                                    psz[par][64:65, :])
                            rcp = work2.tile([1, 1024], F32, tag="rcp", name="rcp")
                            nc.vector.reciprocal_approx_fast(rcp[:], den[:])
                            bc = work2.tile([64, 1024], F32, tag="bc", name="bc")
                            nc.gpsimd.partition_broadcast(bc[:], rcp[:])

                            def part2(bc=bc, psz=psz, t=t, hp=hp):
                                for h2 in range(2):
                                    for par in (0, 1):
                                        c0 = h2 * 256
                                        nc.vector.tensor_mul(
                                            ZT[hp][par * 64:par * 64 + 64,
                                                   t * 512 + c0:t * 512 + c0 + 256],
                                            psz[par][0:64, c0:c0 + 256],
                                            bc[:, par * 512 + c0:par * 512 + c0 + 256])
                            pending.append(part2)
                            return
                        zraw = {}
                        den = work2.tile([1, 1024], F32, tag="den", name="den")
                        for par in (0, 1):
                            zr = workZ.tile([64, 512], BF16, tag="zraw", name="zraw")
                            nc.vector.tensor_copy(zr[:], psz[par][0:64, :])
                            nc.vector.tensor_copy(
                                den[:, par * 512:(par + 1) * 512],
                                psz[par][64:65, :])
                            zraw[par] = zr
                        rcp = work2.tile([1, 1024], F32, tag="rcp", name="rcp")
                        nc.vector.reciprocal_approx_fast(rcp[:], den[:])
                        bc = work2.tile([64, 1024], F32, tag="bc", name="bc")
                        nc.gpsimd.partition_broadcast(bc[:], rcp[:])

                        def part2(zraw=zraw, bc=bc, t=t, hp=hp):
                            for par in (0, 1):
                                nc.vector.tensor_mul(
                                    ZT[hp][par * 64:par * 64 + 64,
                                           t * 512:(t + 1) * 512],
                                    zraw[par][:],
                                    bc[:, par * 512:(par + 1) * 512])
                        pending.append(part2)

                    def make_av(par, et, j, t, q0, psz, hp):
                        def av_fn(par=par, et=et, j=j, t=t, q0=q0, psz=psz,
                                  hp=hp):
                            if j == 0:
                                psz[par] = psZ.tile(
                                    [128, 512], F32, tag=f"pz{par}",
                                    name=f"pz{par}")
                            h = 2 * hp + par
                            nc.tensor.matmul(
                                psz[par][0:65, q0:512],
                                VA[j][:, h * 65:(h + 1) * 65],
                                et[:, par * 512 + q0:par * 512 + 512],
                                start=(j == 0), stop=(j == 4 * t + 3))
                        return av_fn

                    # process js in batches of 2 to reduce PE tiling-mode
                    # switches (QK pairs grouped, then exps, then AVs)
                    js = list(range(4 * t + 4))
                    for jb in range(0, len(js), 2):
                        group = js[jb:jb + 2]
                        last_group = jb + 2 >= len(js)
                        pss_of = {}
                        for j in group:
                            r = j - 4 * t
                            q0 = 128 * r if r >= 0 else 0
                            pss = psS.tile([128, 1024], F32, tag="pss", name="pss")
                            pss_of[j] = (pss, r, q0)
                            for par in (0, 1):
                                nc.tensor.matmul(
                                    pss[:, par * 512 + q0:par * 512 + 512],
                                    KT[hp][par * 64:par * 64 + 64, j * 128:(j + 1) * 128],
                                    QT[hp][par * 64:par * 64 + 64,
                                           t * 512 + q0:(t + 1) * 512],
                                    start=True, stop=True)
                        for j in group:
                            pss, r, q0 = pss_of[j]
                            et = workE.tile([128, 1024], BF16, tag="et", name="et")
                            if r >= 0:
                                src = pss[:].rearrange("p (g q) -> p g q", g=2)[:, :, q0:512]
                                dst = et[:].rearrange("p (g q) -> p g q", g=2)[:, :, q0:512]
                                nc.scalar.activation(dst, src, AF.Exp, scale=0.125)
                                # one batched mask multiply over both heads
                                nc.vector.tensor_mul(
                                    et[:].rearrange("p (g q) -> p g q", g=2)[:, :, q0:q0 + 128],
                                    et[:].rearrange("p (g q) -> p g q", g=2)[:, :, q0:q0 + 128],
                                    MSK2[:].rearrange("p (g q) -> p g q", g=2))
                            else:
                                nc.scalar.activation(et[:], pss[:], AF.Exp, scale=0.125)
                            for par in (0, 1):
                                norm_fn = (emit_normalize
                                           if (j == 4 * t + 3 and par == 1) else None)
                                av_q.append((make_av(par, et, j, t, q0, psz, hp),
                                             norm_fn))
                        if hp == 0:
                            # emitted after the group's QK/exp stream: engine
                            # queues are in-order, so a wv-starved proj_v at
                            # the queue head would block the QKs behind it
                            for j in group:
                                proj_v(j)
                        # pump every third group: AV bursts of 12 amortize
                        # the PE tiling-mode switch better
                        if (jb // 2) % 3 == 2 or last_group:
                            pump_avs(AV_LAG)
                        if jb in (4, 8):
                            # run deferred ZT scales mid-supertile, well
                            # before the next phase_e boundary needs them
                            drain_pending(1)
                        if pe_todo and jb >= 2:
                            phase_e_block(pe_todo.pop(0))
                            if pe_todo:
                                phase_e_block(pe_todo.pop(0))
                        if weave_todo and jb >= 2:
                            weave_todo.pop(0)()
                    while pe_todo:
                        phase_e_block(pe_todo.pop(0))
                    while weave_todo:
                        weave_todo.pop(0)()

            pump_avs(0)
            for mc in held_blocks:
                phase_e_block(mc, act_copy=True)
            drain_pending(0)
            for mc in range(8, 12):
                phase_e_block(mc, act_copy=True)
    nc.compile()
    return nc


def _build_mask2() -> np.ndarray:
    kl = np.arange(128)[:, None]
    ql = np.arange(128)[None, :]
    m = (ql >= kl).astype(np.float32)
    return np.concatenate([m, m], axis=1)


def kernel(**inputs) -> np.ndarray:
    global LAST_EXEC_TIME_NS
    x = np.asarray(inputs["normalized_resid_pre"], dtype=np.float32)
    W_Q = np.asarray(inputs["W_Q"], dtype=np.float32)
    W_K = np.asarray(inputs["W_K"], dtype=np.float32)
    W_V = np.asarray(inputs["W_V"], dtype=np.float32)
    W_O = np.asarray(inputs["W_O"], dtype=np.float32)
    b_Q = np.asarray(inputs["b_Q"], dtype=np.float32)
    b_K = np.asarray(inputs["b_K"], dtype=np.float32)
    b_V = np.asarray(inputs["b_V"], dtype=np.float32)
    b_O = np.asarray(inputs["b_O"], dtype=np.float32)

    qkv_bias = bool(b_Q.any() or b_K.any() or b_V.any())
    key = qkv_bias
    if key not in _GRAPH_CACHE:
        _GRAPH_CACHE[key] = _build_graph(qkv_bias)
    nc = _GRAPH_CACHE[key]

    mask2 = _build_mask2()
    in_maps = []
    for c in range(8):
        b, h0 = c // 2, NHC * (c % 2)
        im = {
            "xt": np.ascontiguousarray(x[b].T).astype(BF16NP),
            "wq": np.ascontiguousarray(
                W_Q[h0:h0 + NHC].transpose(1, 0, 2).reshape(D, HE)).astype(BF16NP),
            "wk": np.ascontiguousarray(
                W_K[h0:h0 + NHC].transpose(1, 0, 2).reshape(D, HE)).astype(BF16NP),
            "wv": np.ascontiguousarray(
                W_V[h0:h0 + NHC].transpose(1, 0, 2).reshape(D, HE)).astype(BF16NP),
            "wo": np.ascontiguousarray(W_O[h0:h0 + NHC].reshape(HE, D)).astype(BF16NP),
            "mask2": mask2.astype(BF16NP),
        }
        if qkv_bias:
            im["bq"] = np.ascontiguousarray(b_Q[h0:h0 + NHC].reshape(HE, 1))
            im["bk"] = np.ascontiguousarray(b_K[h0:h0 + NHC].reshape(HE, 1))
            im["bv"] = np.ascontiguousarray(b_V[h0:h0 + NHC].reshape(1, HE)).astype(BF16NP)
        in_maps.append(im)

    import os
    trace = bool(os.environ.get("KERNEL_TRACE"))
    res = run_bass_kernel_spmd(nc, in_maps, core_ids=list(range(8)), trace=trace)
    LAST_EXEC_TIME_NS = res.exec_time_ns
    results = res.results

    outf = np.empty((B, S, D), dtype=np.float32)
    for b in range(B):
        outf[b] = results[2 * b]["out"].astype(np.float32) + \
            results[2 * b + 1]["out"].astype(np.float32)
    if b_O.any():
        outf += b_O
    return outf
